# revision 3
# baseline (speedup 1.0000x reference)
"""Trainium2 Bass kernel for nn_HardLinearAttention.

Math: out = Z + (alpha/n) * P @ Z @ M @ Z.T @ Q @ Z with
  P = e_last e_last^T, M = lower-tri lambda^(i-j) (last row/col zero),
  Q = [[-I, I],[0,0]] blocks.
Because P has a single nonzero (bottom-right), the update is rank-1: only the
last row of the output differs from Z.  With z = Z[-1,:] (masked at col n):
  r[j] = sum_k lambda^k z[j+k]          (geometric window, 128 taps:
                                         lambda^128 ~ 1.4e-6, below the f32
                                         noise of the reference's dense sum)
  s[i] = sum_j Z[i,j] r[j]   (i < d)    (only s[0:d] survives Q)
  u[j] = sum_k s[k] (Z[d+k,j] - Z[k,j])
  out[-1,:] = Z[-1,:] + (alpha/n) u ;  out[i,:] = Z[i,:] otherwise.

Sharding: context axis (n+1) split 8 ways (1025 cols/core over a zero-padded
8200-wide array).  Each core computes its local r block and partial s, a 2KB
AllGather + local sum combines s across cores, then each core forms u for its
columns and writes its output shard.  Rows 0..1023 are a straight copy routed
through SBUF so one HBM read feeds both the copy and the compute; Ztop is
transposed on-chip (PE) for the j-contraction instead of shipping a second
copy of it through HBM.
"""

import sys

for _p in ("/opt/trn_rl_repo", "/root/.axon_site/_ro/trn_rl_repo"):
    if _p not in sys.path:
        sys.path.append(_p)

import numpy as np

import concourse.bacc as bacc
import concourse.bass as bass
import concourse.mybir as mybir
import concourse.tile as tile
from concourse.ap import AP
from concourse import bass_utils

F32 = mybir.dt.float32

D = 512          # feature dim d
N = 8192         # context length n
R = 2 * D + 1    # 1025 rows
NC = 8           # cores
L = 1025         # columns per core (8 * 1025 = 8200 >= 8193)
WTOT = NC * L    # 8200 padded width
W = 128          # geometric window taps
LPAD = 1152      # local column count padded to 9*128 for full j-tiles
ZWLEN = 1280     # zwin input length: LPAD + W - 1 = 1279, rounded up
NT_J = LPAD // 128   # 9 j-tiles
NT_K = D // 128      # 4 feature tiles
NT_ROW = 8           # full 128-row tiles (rows 0..1023)
J_CHUNKS = [(0, 512), (512, 1024), (1024, 1025)]

_PROGRAM = None


def _build_program():
    nc = bacc.Bacc(
        "TRN2",
        target_bir_lowering=False,
        debug=False,
        enable_asserts=False,
        num_devices=NC,
    )

    zc_d = nc.dram_tensor("zc", [R, L], F32, kind="ExternalInput")
    zwin_d = nc.dram_tensor("zwin", [ZWLEN], F32, kind="ExternalInput")
    lam_d = nc.dram_tensor("lam", [W], F32, kind="ExternalInput")
    alpha_d = nc.dram_tensor("alpha", [1], F32, kind="ExternalInput")
    out_d = nc.dram_tensor("out", [R, L], F32, kind="ExternalOutput")
    ident_d = nc.inline_tensor(np.eye(128, dtype=np.float32), name="ident")

    with tile.TileContext(nc) as tc:
        with (
            tc.tile_pool(name="consts", bufs=1) as consts,
            tc.tile_pool(name="ztp", bufs=3) as ztp,
            tc.tile_pool(name="zrows", bufs=1) as zrows,
            tc.tile_pool(name="work", bufs=1) as work,
            tc.tile_pool(name="rp_ps", bufs=2, space=bass.MemorySpace.PSUM) as rp_ps,
            tc.tile_pool(name="tp_ps", bufs=2, space=bass.MemorySpace.PSUM) as tp_ps,
            tc.tile_pool(name="sc_ps", bufs=2, space=bass.MemorySpace.PSUM) as sc_ps,
            tc.tile_pool(name="u_ps", bufs=2, space=bass.MemorySpace.PSUM) as u_ps,
            tc.tile_pool(name="ccdram", bufs=1, space="DRAM") as ccdram,
        ):
            # ---- critical-path small loads ------------------------------
            lam0 = consts.tile([128, 1], F32, name="lam0")
            nc.sync.dma_start(lam0[:], lam_d[0:W].unsqueeze(1))
            ident = consts.tile([128, 128], F32, name="ident_sb")
            nc.sync.dma_start(ident[:], ident_d[:])

            # overlapping window: win[k, j] = zwin[k + j], 4 parallel chunks
            win = consts.tile([128, LPAD], F32, name="win")
            for q in range(4):
                nc.sync.dma_start(
                    win[q * 32:(q + 1) * 32, :],
                    AP(zwin_d, q * 32, [[1, 32], [1, LPAD]]),
                )

            alpha_sb = consts.tile([1, 1], F32, name="alpha_sb")
            nc.sync.dma_start(alpha_sb[:], alpha_d[0:1].unsqueeze(1))
            scale_sb = consts.tile([1, 1], F32, name="scale_sb")
            nc.vector.tensor_scalar_mul(scale_sb[:], alpha_sb[:], 1.0 / float(N))

            # ---- stage 1: r columns  r[jt] = win_tile.T @ lam -----------
            rcols = work.tile([128, NT_J], F32, name="rcols")
            for t in range(NT_J):
                rp = rp_ps.tile([128, 1], F32, name="rp", tag="rp")
                nc.tensor.matmul(
                    rp[:], win[:, t * 128:(t + 1) * 128], lam0[:],
                    start=True, stop=True,
                )
                nc.vector.tensor_copy(rcols[:, t:t + 1], rp[:])

            # ---- bulk row loads (Ztop first: feeds stage-2 transposes) --
            zv = []
            for t in range(NT_ROW):
                zv_t = zrows.tile([128, L], F32, name=f"zv{t}", tag=f"zv{t}")
                nc.sync.dma_start(zv_t[0:64, :], zc_d[t * 128:t * 128 + 64, :])
                nc.sync.dma_start(zv_t[64:128, :], zc_d[t * 128 + 64:(t + 1) * 128, :])
                zv.append(zv_t)

            # ---- stage 2: on-chip transpose of Ztop feeds s = Ztop @ r --
            # transpose(zv[ic][:, jt]) is exactly the (j, i) lhsT block that
            # the j-contraction needs; accumulate over j-tiles per ic.
            s_sb = work.tile([128, NT_K], F32, name="s_sb")
            for ic in range(NT_K):
                sc = sc_ps.tile([128, 1], F32, name="sc", tag="sc")
                for t in range(NT_J):
                    j0 = t * 128
                    j1 = min((t + 1) * 128, L)
                    jn = j1 - j0
                    tp = tp_ps.tile([128, 128], F32, name="tp", tag="tp")
                    nc.tensor.transpose(tp[0:jn, :], zv[ic][:, j0:j1], ident[:])
                    ztb = ztp.tile([128, 128], F32, name="ztb", tag="ztb")
                    nc.vector.tensor_copy(ztb[0:jn, :], tp[0:jn, :])
                    nc.tensor.matmul(
                        sc[:], ztb[0:jn, :], rcols[0:jn, t:t + 1],
                        start=(t == 0), stop=(t == NT_J - 1),
                    )
                nc.vector.tensor_copy(s_sb[:, ic:ic + 1], sc[:])

            # ---- AllGather partial s (2 KB) + local sum -----------------
            cc_in = ccdram.tile([128, NT_K], F32, name="cc_in")
            cc_out = ccdram.tile([NC * 128, NT_K], F32, name="cc_out")
            nc.gpsimd.dma_start(cc_in[:], s_sb[:])
            nc.gpsimd.collective_compute(
                "AllGather",
                mybir.AluOpType.bypass,
                replica_groups=[list(range(NC))],
                ins=[cc_in.opt()],
                outs=[cc_out.opt()],
            )
            sg = work.tile([128, NC, NT_K], F32, name="sg")
            nc.gpsimd.dma_start(sg[:], cc_out.rearrange("(r p) c -> p r c", p=128))

            # ---- remaining bulk loads + copy-out of rows 0..1023 --------
            for t in range(NT_ROW):
                nc.sync.dma_start(out_d[t * 128:t * 128 + 64, :], zv[t][0:64, :])
                nc.sync.dma_start(out_d[t * 128 + 64:(t + 1) * 128, :], zv[t][64:128, :])
            zlast = zrows.tile([1, L], F32, name="zlast")
            nc.sync.dma_start(zlast[:], zc_d[R - 1:R, :])

            ssum = work.tile([128, NT_K], F32, name="ssum")
            nc.vector.tensor_add(ssum[:], sg[:, 0, :], sg[:, 1, :])
            for r_ in range(2, NC):
                nc.vector.tensor_add(ssum[:], ssum[:], sg[:, r_, :])

            # ---- stage 3: zd = Zmid - Ztop;  u = zd.T @ s ---------------
            zd = []
            for kt in range(NT_K):
                zd_t = work.tile([128, L], F32, name=f"zd{kt}", tag=f"zd{kt}")
                nc.vector.tensor_sub(zd_t[:], zv[NT_K + kt][:], zv[kt][:])
                zd.append(zd_t)

            for (j0, j1) in J_CHUNKS:
                u = u_ps.tile([1, j1 - j0], F32, name="u", tag="u")
                for kt in range(NT_K):
                    nc.tensor.matmul(
                        u[:], ssum[:, kt:kt + 1], zd[kt][:, j0:j1],
                        start=(kt == 0), stop=(kt == NT_K - 1),
                    )
                newrow = work.tile([1, j1 - j0], F32, name="newrow", tag="newrow")
                nc.vector.scalar_tensor_tensor(
                    newrow[:], u[:], scale_sb[:], zlast[:, j0:j1],
                    op0=mybir.AluOpType.mult, op1=mybir.AluOpType.add,
                )
                nc.sync.dma_start(out_d[R - 1:R, j0:j1], newrow[:])

    nc.compile()
    return nc


def _get_program():
    global _PROGRAM
    if _PROGRAM is None:
        _PROGRAM = _build_program()
    return _PROGRAM


def _make_in_maps(Z, alpha, M=None):
    Z = np.asarray(Z, dtype=np.float32)
    alpha = np.asarray(alpha, dtype=np.float32).reshape(1)
    # lambda powers; prefer deriving from M's first column when provided.
    if M is not None:
        lam = np.ascontiguousarray(np.asarray(M)[0:W, 0], dtype=np.float32)
    else:
        lam = (0.9 ** np.arange(W)).astype(np.float32)

    Zp = np.zeros((R, WTOT), dtype=np.float32)
    Zp[:, : N + 1] = Z
    zmpad = np.zeros(WTOT + ZWLEN, dtype=np.float32)
    zmpad[:N] = Z[R - 1, :N]  # col n masked to zero (M's last row is zero)

    in_maps = []
    for c in range(NC):
        j0 = c * L
        in_maps.append(
            {
                "zc": np.ascontiguousarray(Zp[:, j0:j0 + L]),
                "zwin": np.ascontiguousarray(zmpad[j0:j0 + ZWLEN]),
                "lam": lam,
                "alpha": alpha,
            }
        )
    return in_maps


def kernel(Z, alpha, P=None, M=None, Q=None, **_ignored):
    nc = _get_program()
    in_maps = _make_in_maps(Z, alpha, M)
    res = bass_utils.run_bass_kernel_spmd(nc, in_maps, core_ids=list(range(NC)))
    shards = [res.results[c]["out"] for c in range(NC)]
    full = np.concatenate(shards, axis=1)[:, : N + 1]
    return full.astype(np.float32)
